# revision 1
# baseline (speedup 1.0000x reference)
"""Trainium2 Bass kernel v3: fused custom-DVE ops + PE bilinear offload.

cost[q,t] = L1 + (-prob) + (-iou) + pen + alpha*v, per (query, target) pair.

Layout: queries on partitions (7 full tiles of 128 + repacked 4-query tail),
targets on the free axis (T=1600). Per-core batch-parallel over 8 cores.

Key structure vs the fp16 elementwise baseline:
- ox = min(qx2,tx2)-max(qx1,tx1) (signed overlap) via ONE fused DVE op;
  everything (inter, convex width, L1) derives from ox/oy:
    uX = (qw+tw) - 2*ox   (accumulated on PE: bilinear rows + (-2*I)@ox)
    cw = (qw+tw) - ox     (PE K-rows + (-1*I)@ox -> psum; ACT squares it)
    inter4 = relu(2*ox)*relu(2*oy) (Pool)
- All three divisions are single fused DVE ops (bitwise-not seed + 1 Newton
  + numerator multiply), ~2e-3 relative.
- union, convex-width, L1, class terms ride the PE as K-row matmuls.
- ACT does squares (convex, v, vsq) and the PSUM->SBUF output copy; Pool
  does inter/diag/adn; tail (last 4 queries) is computed at [128, 50] with
  its prep hoisted before the main loop so it interleaves.
"""

import math
from contextlib import ExitStack

import numpy as np

import concourse.bass as bass
import concourse.bacc as bacc
import concourse.mybir as mybir
import concourse.tile as tile
import concourse.dve_ops as dve_ops
from concourse.bass_utils import run_bass_kernel_spmd
from concourse.dve_ops import DveOp, RECIP_APPROX_FAST_CONSTS
from concourse.dve_spec import (
    AluOp, Bin, C0, C1, C2, Spec, Src0, Src1, lower,
    _has_src1 as _hs1, maxx, minn, relu, sq,
)
from concourse.dve_uop import DveOpSpec
from concourse.masks import make_identity

B, Q, C, T = 8, 900, 92, 1600
REPEAT = 1
KVER = 42
EPS = 1e-6
ADEN_EPS = 1e-4  # keeps the alpha denominator fp16-normal; error << tolerance
P = 128
NQT = (Q + P - 1) // P
NFULL = NQT - 1
F32 = mybir.dt.float32
F16 = mybir.dt.float16
I32 = mybir.dt.int32
AF = mybir.ActivationFunctionType
OP = mybir.AluOpType
AX = mybir.AxisListType

N_CHUNKS = [(0, 512), (512, 1024), (1024, 1536), (1536, 1600)]
HALF = 800
HALVES = [(0, HALF), (HALF, T)]
H_CHUNKS = [(0, 512), (512, 800)]
TQ, TC, TW = 4, 32, 50
TWP = 64
Q0 = Q - TQ

CH0 = RECIP_APPROX_FAST_CONSTS["s0"]
CH1 = RECIP_APPROX_FAST_CONSTS["s1"]

# ---------------- custom DVE op registration (idempotent) -------------------


def _np_recip1(x, c0, c1):
    x32 = np.asarray(x, np.float32)
    nx = (~x32.view(np.int32)).view(np.float32)
    y0 = nx * c0
    return y0 * (c1 - x32 * y0)


def _recip1(x):
    nx = Bin(AluOp.BITWISE_NOT, x, x)
    y0 = nx * C0
    return y0 * (C1 - x * y0)


def _mk_op(name, body, reference):
    spec = Spec(body=body, reference=reference)
    ver = "v3"
    tmp = DveOpSpec(name=name, opcode=1, uops=lower(spec, ver=ver), rd1_en=_hs1(spec))
    return DveOp(name, spec, False, {ver: tmp.sha(ver)})


_OPS = {}


def _register_ops():
    global OX_ANT, CD_ANT, DIVS_ANT, DIVA_ANT
    defs = [
        ("OX_ANT3", minn(Src0, C0) - maxx(Src1, C1),
         lambda in0, in1, s0, s1, imm2: np.minimum(in0, s0) - np.maximum(in1, s1)),
        ("CD_ANT3", (sq(Src0 - C0) + sq(Src1 - C1)) * C2,
         lambda in0, in1, s0, s1, imm2: ((in0 - s0) ** 2 + (in1 - s1) ** 2) * imm2),
        ("DIVS_ANT3", Src1 * _recip1(Src0) * C2,
         lambda in0, in1, s0, s1, imm2: in1 * _np_recip1(in0, s0, s1) * imm2),
        ("DIVA_ANT3", Src1 * _recip1(Src0 + C2),
         lambda in0, in1, s0, s1, imm2: in1 * _np_recip1(in0 + imm2, s0, s1)),
    ]
    for name, body, ref in defs:
        if name in dve_ops._SUB_OPCODE_FOR_NAME:
            _OPS[name] = next(o for o in dve_ops.OPS if o.name == name)
            continue
        op = _mk_op(name, body, ref)
        row = dve_ops._CUSTOM_DVE_ROW_BASE + len(dve_ops.OPS)
        assert row < 0x20, "custom-DVE 5-bit row overflow"
        dve_ops.OPS.append(op)
        dve_ops._SUB_OPCODE_FOR_NAME[name] = row
        dve_ops.CUSTOM_DVE_SPECS[name] = op.spec
        _OPS[name] = op
    OX_ANT = _OPS["OX_ANT3"]
    CD_ANT = _OPS["CD_ANT3"]
    DIVS_ANT = _OPS["DIVS_ANT3"]
    DIVA_ANT = _OPS["DIVA_ANT3"]


_register_ops()

# ---------------------------------------------------------------------------


def _bcast_ap(ap, npart, inner_ap):
    return bass.AP(tensor=ap.tensor, offset=ap.offset, ap=[[0, npart]] + inner_ap)


def emit_atan(nc, dst, wt, ht, mkt):
    """atan(wt/(ht+eps)) with range reduction; mkt() yields scratch tiles."""
    t1 = mkt()
    nc.vector.tensor_scalar(out=t1, in0=ht, scalar1=EPS, scalar2=None, op0=OP.add)
    t2 = mkt()
    nc.vector.reciprocal_approx_fast(out=t2, in_=t1)
    r = mkt()
    nc.vector.tensor_tensor(out=r, in0=wt, in1=t2, op=OP.mult)
    ri = mkt()
    nc.vector.reciprocal_approx_fast(out=ri, in_=r)
    rc = mkt()
    nc.vector.tensor_tensor(out=rc, in0=r, in1=ri, op=OP.min)
    atc = mkt()
    nc.scalar.activation(out=atc, in_=rc, func=AF.Arctan)
    m = mkt()
    nc.vector.tensor_scalar(out=m, in0=r, scalar1=1.0, scalar2=None, op0=OP.is_gt)
    t3 = mkt()
    nc.vector.tensor_scalar(
        out=t3, in0=atc, scalar1=-2.0, scalar2=math.pi / 2.0, op0=OP.mult, op1=OP.add
    )
    mt = mkt()
    nc.vector.tensor_tensor(out=mt, in0=m, in1=t3, op=OP.mult)
    nc.vector.tensor_tensor(out=dst, in0=atc, in1=mt, op=OP.add)


def build_kernel():
    nc = bacc.Bacc()

    logits_h = nc.declare_dram_parameter("logits", [Q, C], F32, isOutput=False)
    qbox_h = nc.declare_dram_parameter("qbox", [Q, 4], F32, isOutput=False)
    tbox_h = nc.declare_dram_parameter("tbox", [T, 4], F32, isOutput=False)
    tid_h = nc.declare_dram_parameter("tid", [T], I32, isOutput=False)
    nc.declare_dram_parameter("rep_marker", [KVER + REPEAT], F32, isOutput=False)
    out_h = nc.declare_dram_parameter("out", [Q, T], F16, isOutput=True)

    with ExitStack() as ctx:
        tc = ctx.enter_context(tile.TileContext(nc))
        consts = ctx.enter_context(tc.tile_pool(name="consts", bufs=1))
        rows = ctx.enter_context(tc.tile_pool(name="rows", bufs=1))
        qcols = ctx.enter_context(tc.tile_pool(name="qcols", bufs=1))
        tailp = ctx.enter_context(tc.tile_pool(name="tailp", bufs=1))

        # ---------------- constants ----------------
        ident_h = consts.tile([P, P], F16, tag="ident_h")
        make_identity(nc, ident_h)
        identm1 = consts.tile([P, P], F16, tag="identm1")
        make_identity(nc, identm1)
        nc.vector.tensor_scalar(out=identm1, in0=identm1, scalar1=-1.0, scalar2=None, op0=OP.mult)
        identm2 = consts.tile([P, P], F16, tag="identm2")
        make_identity(nc, identm2)
        nc.vector.tensor_scalar(out=identm2, in0=identm2, scalar1=-2.0, scalar2=None, op0=OP.mult)
        ic_i = consts.tile([C, 1], I32, tag="ic_i")
        nc.gpsimd.iota(ic_i, pattern=[[0, 1]], base=0, channel_multiplier=1)
        ic_f = consts.tile([C, 1], F32, tag="ic_f")
        nc.vector.tensor_copy(ic_f, ic_i)

        # persistent row tiles
        # R01X: onehot rows 0..91, row 92 = ones, row 93 = tw+th (L1 bilinear)
        R01X = rows.tile([C + 2, T], F16, tag="R01X")
        # rows96: K2-group rhs rows at legal matmul base partitions:
        #   0: -Ra4, 1: ones (nun) | 32: Rw, 33: ones (dgx) | 64: Rh, 65: ones (dgy)
        rows96 = rows.tile([66, T], F16, tag="rows96")
        traw = rows.tile([P, T, 4], F32, tag="traw")
        Rcxb = rows.tile([P, T], F16, tag="Rcxb")
        Rcyb = rows.tile([P, T], F16, tag="Rcyb")
        Ab = rows.tile([P, T], F16, tag="Ab")

        eTX = qcols.tile([C + 2, NQT, P], F16, tag="eTX")
        slabT = qcols.tile([66, NQT, P], F16, tag="slabT")

        # tail persistent tiles (prep written before main loop, read after)
        trawt = tailp.tile([P, TW, 4], F32, tag="trawt")
        Rw32t = tailp.tile([P, TW], F32, tag="Rw32t")
        Rh32t = tailp.tile([P, TW], F32, tag="Rh32t")
        RwHt = tailp.tile([P, TW], F16, tag="RwHt")
        RhHt = tailp.tile([P, TW], F16, tag="RhHt")
        Ra4t = tailp.tile([P, TW], F32, tag="Ra4t")
        Rcxt = tailp.tile([P, TW], F32, tag="Rcxt")
        Rcyt = tailp.tile([P, TW], F32, tag="Rcyt")
        Ratt = tailp.tile([P, TW], F32, tag="Ratt")
        tqb = tailp.tile([P, 4], F32, tag="tqb")
        g50 = tailp.tile([P, TW], F32, tag="g50")

        def ttile(tag):
            return tailp.tile([P, 1], F32, tag=tag, name=tag)

        tqw = ttile("tqw")
        tqh = ttile("tqh")
        tnqx1 = ttile("tnqx1")
        tnqy1 = ttile("tnqy1")
        tnqx2 = ttile("tnqx2")
        tnqy2 = ttile("tnqy2")
        tqa4e = ttile("tqa4e")
        tnqcx = ttile("tnqcx")
        tnqcy = ttile("tnqcy")
        tqat = ttile("tqat")
        tnqat = ttile("tnqat")

        mtiles = ctx.enter_context(tc.tile_pool(name="mtiles", bufs=2))
        ostage = ctx.enter_context(tc.tile_pool(name="ostage", bufs=2))

        with tc.tile_pool(name="scratch", bufs=1) as scratch, tc.tile_pool(
            name="prep_psum", bufs=2, space="PSUM"
        ) as ppsum:
            # ---- input DMAs issued first ----
            nc.sync.dma_start(
                out=traw[:, :, :], in_=_bcast_ap(tbox_h[:, :], P, [[4, T], [1, 4]])
            )
            tid_i = scratch.tile([C, T], I32, tag="tid_i")
            nc.gpsimd.dma_start(out=tid_i[:, :], in_=_bcast_ap(tid_h[:], C, [[1, T]]))
            CP, CW = 100, 16
            ctb = scratch.tile([P, CW, 4], F32, tag="ctb")
            nc.vector.memset(ctb, 1.0)
            nc.vector.memset(ctb[:, :, 0:2], 0.25)
            nc.gpsimd.dma_start(
                out=ctb[0:CP, :, :],
                in_=bass.AP(
                    tensor=tbox_h[:, :].tensor,
                    offset=tbox_h[:, :].offset,
                    ap=[[CW * 4, CP], [4, CW], [1, 4]],
                ),
            )
            qb = qcols.tile([P, NQT, 4], F32, tag="qb")
            nc.vector.memset(qb, 1.0)
            nc.vector.memset(qb[:, :, 0:2], 0.25)
            nfull = Q // P
            nc.gpsimd.dma_start(
                out=qb[:, 0:nfull, :],
                in_=bass.AP(
                    tensor=qbox_h[:, :].tensor,
                    offset=qbox_h[:, :].offset,
                    ap=[[4, P], [P * 4, nfull], [1, 4]],
                ),
            )
            nc.gpsimd.dma_start(out=qb[0 : Q - nfull * P, nfull, :], in_=qbox_h[nfull * P : Q, :])
            for q in range(TQ):
                nc.gpsimd.dma_start(
                    out=tqb[q * TC : (q + 1) * TC, :],
                    in_=bass.AP(
                        tensor=qbox_h[:, :].tensor,
                        offset=qbox_h[:, :].offset + (Q0 + q) * 4,
                        ap=[[0, TC], [1, 4]],
                    ),
                )
                nc.gpsimd.dma_start(
                    out=trawt[q * TC : (q + 1) * TC, :, :],
                    in_=bass.AP(
                        tensor=tbox_h[:, :].tensor,
                        offset=tbox_h[:, :].offset,
                        ap=[[TW * 4, TC], [4, TW], [1, 4]],
                    ),
                )

            # ---- softmax phase A (Exp table first) ----
            mneg8 = qcols.tile([P, NQT], F32, tag="mneg8")
            ssum8 = qcols.tile([P, NQT], F32, tag="ssum8")
            nc.vector.memset(ssum8, 1.0)
            e_all = qcols.tile([P, NQT, C], F32, tag="e_all")
            for k in [NQT - 1] + list(range(NFULL)):
                pk = min(P, Q - k * P)
                L = scratch.tile([P, C], F32, tag="L", name="L", bufs=3)
                nc.gpsimd.dma_start(out=L[0:pk, :], in_=logits_h[k * P : k * P + pk, :])
                nc.vector.tensor_reduce(
                    out=mneg8[0:pk, k : k + 1], in_=L[0:pk, :], axis=AX.X, op=OP.max,
                    negate=True,
                )
                nc.scalar.activation(
                    out=e_all[0:pk, k, :], in_=L[0:pk, :], func=AF.Exp,
                    bias=mneg8[0:pk, k : k + 1], scale=1.0,
                    accum_out=ssum8[0:pk, k : k + 1],
                )
            nr8 = qcols.tile([P, NQT], F32, tag="nr8")
            nc.vector.reciprocal(out=nr8, in_=ssum8)
            nc.vector.tensor_scalar(
                out=nr8, in0=nr8, scalar1=-1.0, scalar2=None, op0=OP.mult
            )

            # ---- onehot ----
            nc.vector.tensor_scalar(
                out=R01X[0:C, :], in0=tid_i, scalar1=ic_f[:, 0:1], scalar2=None,
                op0=OP.is_equal,
            )

            # ---- compact target rows ----
            def cs32(tag):
                return scratch.tile([P, CW], F32, tag=tag, name=tag)

            def cs16(tag):
                return scratch.tile([P, CW], F16, tag=tag, name=tag)

            cRw = cs32("cRw")
            cRh = cs32("cRh")
            nc.vector.tensor_tensor(out=cRw, in0=ctb[:, :, 2], in1=ctb[:, :, 0], op=OP.subtract)
            nc.vector.tensor_tensor(out=cRh, in0=ctb[:, :, 3], in1=ctb[:, :, 1], op=OP.subtract)
            c_twh = cs16("c_twh")
            nc.vector.tensor_tensor(out=c_twh, in0=cRw, in1=cRh, op=OP.add)
            c_nRa4 = cs16("c_nRa4")
            nc.vector.scalar_tensor_tensor(
                out=c_nRa4, in0=cRw, scalar=-4.0, in1=cRh, op0=OP.mult, op1=OP.mult
            )
            c_Rw = cs16("c_Rw")
            c_Rh = cs16("c_Rh")
            nc.vector.tensor_copy(c_Rw, cRw)
            nc.vector.tensor_copy(c_Rh, cRh)
            c_ones = cs16("c_ones")
            nc.vector.memset(c_ones, 1.0)
            c_Rcx = cs16("c_Rcx")
            c_Rcy = cs16("c_Rcy")
            nc.vector.tensor_tensor(out=c_Rcx, in0=ctb[:, :, 0], in1=ctb[:, :, 2], op=OP.add)
            nc.vector.tensor_tensor(out=c_Rcy, in0=ctb[:, :, 1], in1=ctb[:, :, 3], op=OP.add)
            cAt = cs32("cAt")
            _atc = [0]

            def _mka():
                _atc[0] += 1
                return scratch.tile([P, CW], F32, tag="att", name="att", bufs=5)

            emit_atan(nc, cAt, cRw, cRh, _mka)
            c_A = cs16("c_A")
            nc.vector.tensor_scalar(
                out=c_A, in0=cAt, scalar1=2.0 / math.pi, scalar2=None, op0=OP.mult
            )

            # ---- per-query scalars ----
            qx1 = qb[:, :, 0]
            qy1 = qb[:, :, 1]
            qx2 = qb[:, :, 2]
            qy2 = qb[:, :, 3]

            def qt(tag):
                return qcols.tile([P, NQT], F32, tag=tag, name=tag)

            qw8 = qt("qw8")
            qh8 = qt("qh8")
            nc.vector.tensor_tensor(out=qw8, in0=qx2, in1=qx1, op=OP.subtract)
            nc.vector.tensor_tensor(out=qh8, in0=qy2, in1=qy1, op=OP.subtract)
            qcx8 = qt("qcx8")
            qcy8 = qt("qcy8")
            nc.vector.tensor_tensor(out=qcx8, in0=qx1, in1=qx2, op=OP.add)
            nc.vector.tensor_tensor(out=qcy8, in0=qy1, in1=qy2, op=OP.add)
            na8 = qt("na8")
            qat8 = qt("qat8")
            _qtc = [0]

            def _mkq():
                _qtc[0] += 1
                return qcols.tile([P, NQT], F32, tag="qat_t", name="qat_t", bufs=5)

            emit_atan(nc, qat8, qw8, qh8, _mkq)
            nc.vector.tensor_scalar(
                out=na8, in0=qat8, scalar1=-2.0 / math.pi, scalar2=None, op0=OP.mult
            )

            # ---- tail scalar prep + atans (Arctan table stays prep-only) ----
            _tat_tiles = [ttile(f"tat{i}") for i in range(9)]
            nc.vector.tensor_tensor(out=tqw, in0=tqb[:, 2:3], in1=tqb[:, 0:1], op=OP.subtract)
            nc.vector.tensor_tensor(out=tqh, in0=tqb[:, 3:4], in1=tqb[:, 1:2], op=OP.subtract)
            for dst, src in (
                (tnqx1, tqb[:, 0:1]), (tnqy1, tqb[:, 1:2]),
                (tnqx2, tqb[:, 2:3]), (tnqy2, tqb[:, 3:4]),
            ):
                nc.vector.tensor_scalar(out=dst, in0=src, scalar1=-1.0, scalar2=None, op0=OP.mult)
            nc.vector.scalar_tensor_tensor(
                out=tqa4e, in0=tqw, scalar=4.0, in1=tqh, op0=OP.mult, op1=OP.mult
            )
            nc.vector.tensor_scalar(
                out=tqa4e, in0=tqa4e, scalar1=4.0 * EPS, scalar2=None, op0=OP.add
            )
            nc.vector.scalar_tensor_tensor(
                out=tnqcx, in0=tqb[:, 0:1], scalar=-1.0, in1=tqb[:, 2:3], op0=OP.mult, op1=OP.subtract
            )
            nc.vector.scalar_tensor_tensor(
                out=tnqcy, in0=tqb[:, 1:2], scalar=-1.0, in1=tqb[:, 3:4], op0=OP.mult, op1=OP.subtract
            )
            _ttc = [0]

            def _mkt1():
                t = _tat_tiles[_ttc[0]]
                _ttc[0] += 1
                return t

            emit_atan(nc, tqat, tqw, tqh, _mkt1)
            nc.vector.tensor_scalar(
                out=tnqat, in0=tqat, scalar1=-2.0 / math.pi, scalar2=None, op0=OP.mult
            )
            ttx1 = trawt[:, :, 0]
            tty1 = trawt[:, :, 1]
            ttx2 = trawt[:, :, 2]
            tty2 = trawt[:, :, 3]
            nc.vector.tensor_tensor(out=Rw32t, in0=ttx2, in1=ttx1, op=OP.subtract)
            nc.vector.tensor_tensor(out=Rh32t, in0=tty2, in1=tty1, op=OP.subtract)
            nc.vector.tensor_copy(RwHt[:, :], Rw32t[:, :])
            nc.vector.tensor_copy(RhHt[:, :], Rh32t[:, :])
            nc.vector.scalar_tensor_tensor(
                out=Ra4t, in0=Rw32t, scalar=4.0, in1=Rh32t, op0=OP.mult, op1=OP.mult
            )
            nc.vector.tensor_tensor(out=Rcxt, in0=ttx1, in1=ttx2, op=OP.add)
            nc.vector.tensor_tensor(out=Rcyt, in0=tty1, in1=tty2, op=OP.add)
            _ttc2 = [0]

            def _mkt2():
                t = tailp.tile([P, TW], F32, tag="attw", name="attw", bufs=5)
                return t[0:P, 0:TW]

            emit_atan(nc, Ratt, Rw32t, Rh32t, _mkt2)

            # ---- DRAM bounces ----
            def bounce(cname, ctile):
                drow = nc.dram_tensor(cname, [T], F16)
                nc.sync.dma_start(out=drow[:], in_=ctile[0:CP, :])
                return drow

            for gname, base, c0t, c1t in (
                ("dg_nun", 0, c_nRa4, c_ones),
                ("dg_dgx", 32, c_Rw, c_ones),
                ("dg_dgy", 64, c_Rh, c_ones),
            ):
                dgrp = nc.dram_tensor(gname, [2, T], F16)
                nc.sync.dma_start(out=dgrp[0:1, :], in_=c0t[0:CP, :])
                nc.sync.dma_start(out=dgrp[1:2, :], in_=c1t[0:CP, :])
                nc.sync.dma_start(out=rows96[base : base + 2, :], in_=dgrp[:, :])
            dbil = nc.dram_tensor("dg_bil", [2, T], F16)
            nc.sync.dma_start(out=dbil[0:1, :], in_=c_ones[0:CP, :])
            nc.sync.dma_start(out=dbil[1:2, :], in_=c_twh[0:CP, :])
            nc.sync.dma_start(out=R01X[C : C + 2, :], in_=dbil[:, :])
            for (cname, ctile, btile), eng in zip((
                ("d_Rcx", c_Rcx, Rcxb),
                ("d_Rcy", c_Rcy, Rcyb),
                ("d_A", c_A, Ab),
            ), (nc.sync, nc.gpsimd, nc.sync)):
                drow = nc.dram_tensor(cname, [T], F16)
                eng.dma_start(out=drow[:], in_=ctile[0:CP, :])
                eng.dma_start(out=btile[:, :], in_=_bcast_ap(drow[:], P, [[1, T]]))

            # ---- per-tile scalar slab: [128, 66] -> transpose -> slabT ----
            qwqh8 = qt("qwqh8")
            nc.vector.tensor_tensor(out=qwqh8, in0=qw8, in1=qh8, op=OP.add)
            nqa4e8 = qt("nqa4e8")
            nc.vector.scalar_tensor_tensor(
                out=nqa4e8, in0=qw8, scalar=-4.0, in1=qh8, op0=OP.mult, op1=OP.mult
            )
            nc.vector.tensor_scalar(
                out=nqa4e8, in0=nqa4e8, scalar1=-4.0 * EPS, scalar2=None, op0=OP.add
            )
            for k in range(NQT):
                pk = min(P, Q - k * P)
                sl66 = scratch.tile([P, 66], F16, tag="sl66", name="sl66", bufs=2)
                nc.vector.memset(sl66, 0.0)
                nc.vector.memset(sl66[:, 0:1], 1.0)
                nc.vector.tensor_copy(sl66[:, 1:2], nqa4e8[:, k : k + 1])
                nc.vector.memset(sl66[:, 32:33], 1.0)
                nc.vector.tensor_copy(sl66[:, 33:34], qw8[:, k : k + 1])
                nc.vector.memset(sl66[:, 64:65], 1.0)
                nc.vector.tensor_copy(sl66[:, 65:66], qh8[:, k : k + 1])
                tp66 = ppsum.tile([66, P], F16, tag="tp66", name="tp66", bufs=2)
                nc.tensor.transpose(tp66[:, 0:pk], sl66[0:pk, :], ident_h[0:pk, 0:pk])
                nc.scalar.copy(out=slabT[:, k, 0:pk], in_=tp66[:, 0:pk])

            # ---- softmax phase B ----
            for k in [NQT - 1] + list(range(NFULL)):
                pk = min(P, Q - k * P)
                es = scratch.tile([P, C + 2], F16, tag="es", name="es", bufs=2)
                nc.vector.tensor_scalar(
                    out=es[0:pk, 0:C], in0=e_all[0:pk, k, :],
                    scalar1=nr8[0:pk, k : k + 1], scalar2=None, op0=OP.mult,
                )
                nc.vector.tensor_copy(es[0:pk, C : C + 1], qwqh8[0:pk, k : k + 1])
                nc.vector.memset(es[0:pk, C + 1 : C + 2], 1.0)
                tp = ppsum.tile([C + 2, P], F16, tag="tp", name="tp")
                nc.tensor.transpose(tp[:, 0:pk], es[0:pk, :], ident_h[0:pk, 0:pk])
                nc.scalar.copy(out=eTX[:, k, 0:pk], in_=tp[:, 0:pk])

            # ---- tail class term: [4, 1600] matmul -> DRAM -> [128, 50] ----
            g4 = ppsum.tile([P, T], F32, tag="g4", name="g4", bufs=1)
            for n0, n1 in N_CHUNKS:
                nc.tensor.matmul(
                    g4[0:TQ, n0:n1], lhsT=eTX[0:C, NFULL, 0:TQ], rhs=R01X[0:C, n0:n1],
                    start=True, stop=True,
                )
            gst = scratch.tile([P, T], F32, tag="gst")
            nc.scalar.copy(out=gst[0:TQ, :], in_=g4[0:TQ, :])
            gdram = nc.dram_tensor("tail_g", [TQ, T], F32)
            nc.gpsimd.dma_start(out=gdram[:, :], in_=gst[0:TQ, :])
            for q in range(TQ):
                nc.gpsimd.dma_start(
                    out=g50[q * TC : (q + 1) * TC, 0:TW],
                    in_=bass.AP(
                        tensor=gdram[:, :].tensor,
                        offset=gdram[:, :].offset + q * T,
                        ap=[[TW, TC], [1, TW]],
                    ),
                )

        # ---------------- main loop ----------------
        gpsum = ctx.enter_context(tc.tile_pool(name="gpsum", bufs=1, space="PSUM"))
        spsum = ctx.enter_context(tc.tile_pool(name="spsum", bufs=2, space="PSUM"))

        def mt16(tag):
            return mtiles.tile([P, T], F16, tag=tag, name=tag)

        def emit_tail():
            with tc.tile_pool(name="ttmp16", bufs=14) as ttmp16, tc.tile_pool(
                name="ttmp32", bufs=7
            ) as ttmp32, tc.tile_pool(name="tadd", bufs=6) as tadd:
                ttx1 = trawt[:, :, 0]
                tty1 = trawt[:, :, 1]
                ttx2 = trawt[:, :, 2]
                tty2 = trawt[:, :, 3]

                def t16(a, b, op, tg="t16"):
                    o = ttmp16.tile([P, TWP], F16, tag=tg, name=tg)
                    nc.vector.tensor_tensor(out=o[:, 0:TW], in0=a, in1=b, op=op)
                    return o[:, 0:TW]

                def act16(in_, func, bias=0.0, scale=1.0):
                    o = ttmp16.tile([P, TWP], F16, tag="a16", name="a16")
                    nc.scalar.activation(out=o[:, 0:TW], in_=in_, func=func, bias=bias, scale=scale)
                    return o[:, 0:TW]

                def t32(tag):
                    return ttmp32.tile([P, TWP], F32, tag="t32", name=tag)

                adx1 = act16(ttx1, AF.Abs, bias=tnqx1)
                adx2 = act16(ttx2, AF.Abs, bias=tnqx2)
                uX = t16(adx1, adx2, OP.add, tg="lng")
                ady1 = act16(tty1, AF.Abs, bias=tnqy1)
                ady2 = act16(tty2, AF.Abs, bias=tnqy2)
                uY = t16(ady1, ady2, OP.add, tg="lng")
                sxw = t16(RwHt[:, :], uX, OP.subtract)
                px = act16(sxw, AF.Relu, bias=tqw)
                syw = t16(RhHt[:, :], uY, OP.subtract)
                py = act16(syw, AF.Relu, bias=tqh)
                inter4t = t16(px, py, OP.mult)
                nun = t32("nun")
                nc.vector.scalar_tensor_tensor(
                    out=nun[:, 0:TW], in0=inter4t, scalar=tqa4e, in1=Ra4t,
                    op0=OP.subtract, op1=OP.subtract,
                )
                rnu = t32("rnu")
                nc.vector.reciprocal_approx_fast(out=rnu[:, 0:TW], in_=nun[:, 0:TW])
                niout = tadd.tile([P, TWP], F16, tag="ad", name="niout")
                nc.vector.tensor_tensor(out=niout[:, 0:TW], in0=inter4t, in1=rnu[:, 0:TW], op=OP.mult)
                cwx = t16(RwHt[:, :], uX, OP.add)
                sqcw = act16(cwx, AF.Square, bias=tqw)
                cwy = t16(RhHt[:, :], uY, OP.add)
                sqch = act16(cwy, AF.Square, bias=tqh)
                diagt = t32("diagt")
                nc.vector.scalar_tensor_tensor(
                    out=diagt[:, 0:TW], in0=sqcw, scalar=4.0 * EPS, in1=sqch,
                    op0=OP.add, op1=OP.add,
                )
                rd = t32("rd")
                nc.vector.reciprocal_approx_fast(out=rd[:, 0:TW], in_=diagt[:, 0:TW])
                ex = act16(Rcxt[:, :], AF.Square, bias=tnqcx)
                ey = act16(Rcyt[:, :], AF.Square, bias=tnqcy)
                cd4t = t16(ex, ey, OP.add)
                pent = tadd.tile([P, TWP], F16, tag="ad", name="pent")
                nc.vector.tensor_tensor(out=pent[:, 0:TW], in0=cd4t, in1=rd[:, 0:TW], op=OP.mult)
                vt = act16(Ratt[:, :], AF.Square, bias=tnqat, scale=2.0 / math.pi)
                adent = t32("adent")
                nc.vector.scalar_tensor_tensor(
                    out=adent[:, 0:TW], in0=niout[:, 0:TW], scalar=1.0 + EPS, in1=vt,
                    op0=OP.add, op1=OP.add,
                )
                ra = t32("ra")
                nc.vector.reciprocal_approx_fast(out=ra[:, 0:TW], in_=adent[:, 0:TW])
                vsqt = act16(vt, AF.Square)
                avt = tadd.tile([P, TWP], F16, tag="ad", name="avt")
                nc.vector.tensor_tensor(out=avt[:, 0:TW], in0=vsqt, in1=ra[:, 0:TW], op=OP.mult)

                # f32 accumulation on DVE (tiny at fd=50)
                s1 = t32("s1")
                nc.vector.tensor_tensor(out=s1[:, 0:TW], in0=niout[:, 0:TW], in1=pent[:, 0:TW], op=OP.add)
                s2 = t32("s2")
                nc.vector.tensor_tensor(out=s2[:, 0:TW], in0=s1[:, 0:TW], in1=avt[:, 0:TW], op=OP.add)
                s3 = t32("s3")
                nc.vector.tensor_tensor(out=s3[:, 0:TW], in0=s2[:, 0:TW], in1=uX, op=OP.add)
                s4 = t32("s4")
                nc.vector.tensor_tensor(out=s4[:, 0:TW], in0=s3[:, 0:TW], in1=uY, op=OP.add)
                ostt = tailp.tile([P, TWP], F16, tag="ostt")
                nc.vector.tensor_tensor(
                    out=ostt[:, 0:TW], in0=g50[:, 0:TW], in1=s4[:, 0:TW], op=OP.add
                )
                for q in range(TQ):
                    nc.gpsimd.dma_start(
                        out=bass.AP(
                            tensor=out_h[:, :].tensor,
                            offset=out_h[:, :].offset + (Q0 + q) * T,
                            ap=[[TW, TC], [1, TW]],
                        ),
                        in_=ostt[q * TC : (q + 1) * TC, 0:TW],
                    )




        _tail_emitted = [False]
        for k in [kk for _rep in range(REPEAT) for kk in range(NFULL)]:
            if k == 4 and not _tail_emitted[0]:
                emit_tail()
                _tail_emitted[0] = True
            pk = P
            sl = slice(k, k + 1)

            ox = mtiles.tile([P, T], F16, tag="ox", name="ox", bufs=3)
            nc.vector._custom_dve(
                OX_ANT, out=ox[0:pk, :], in0=traw[0:pk, :, 2], in1=traw[0:pk, :, 0],
                s0=qb[0:pk, k, 2:3], s1=qb[0:pk, k, 0:1],
            )
            oy = mtiles.tile([P, T], F16, tag="oy", name="oy", bufs=3)
            nc.vector._custom_dve(
                OX_ANT, out=oy[0:pk, :], in0=traw[0:pk, :, 3], in1=traw[0:pk, :, 1],
                s0=qb[0:pk, k, 3:4], s1=qb[0:pk, k, 1:2],
            )
            # inter4 = relu(2*ox)*relu(2*oy) on Pool
            pxp = mt16("pxp")
            nc.gpsimd.tensor_scalar(
                out=pxp[0:pk, :], in0=ox[0:pk, :], scalar1=0.0, scalar2=2.0,
                op0=OP.max, op1=OP.mult,
            )
            pyp = mt16("pyp")
            nc.gpsimd.tensor_scalar(
                out=pyp[0:pk, :], in0=oy[0:pk, :], scalar1=0.0, scalar2=2.0,
                op0=OP.max, op1=OP.mult,
            )
            inter4 = mt16("inter4")
            nc.gpsimd.tensor_tensor(
                out=inter4[0:pk, :], in0=pxp[0:pk, :], in1=pyp[0:pk, :], op=OP.mult
            )

            # --- PE: g accumulation (class + bilinear L1 + -2ox -2oy) ---
            g = gpsum.tile([P, T], F32, tag="g", name="g")
            for n0, n1 in N_CHUNKS:
                nc.tensor.matmul(g[0:pk, n0:n1], lhsT=eTX[:, k, 0:pk], rhs=R01X[:, n0:n1],
                                 start=True, stop=False)
            for n0, n1 in N_CHUNKS:
                nc.tensor.matmul(g[0:pk, n0:n1], lhsT=identm2[0:pk, 0:pk], rhs=ox[0:pk, n0:n1],
                                 start=False, stop=False)
            for n0, n1 in N_CHUNKS:
                nc.tensor.matmul(g[0:pk, n0:n1], lhsT=identm2[0:pk, 0:pk], rhs=oy[0:pk, n0:n1],
                                 start=False, stop=False)

            # --- PE stream psums + consumers, per half ---
            niou = mt16("niou")
            sqx = mt16("sqx")
            sqy = mt16("sqy")
            for h0, h1 in HALVES:
                nun_ps = spsum.tile([P, HALF], F32, tag="stream", name="nun_ps")
                for c0, c1 in H_CHUNKS:
                    nc.tensor.matmul(nun_ps[0:pk, c0:c1], lhsT=ident_h[0:pk, 0:pk],
                                     rhs=inter4[0:pk, h0 + c0 : h0 + c1],
                                     start=True, stop=False)
                    nc.tensor.matmul(nun_ps[0:pk, c0:c1], lhsT=slabT[0:2, k, 0:pk],
                                     rhs=rows96[0:2, h0 + c0 : h0 + c1],
                                     start=False, stop=True)
                nc.vector._custom_dve(
                    DIVS_ANT, out=niou[0:pk, h0:h1], in0=nun_ps[0:pk, :],
                    in1=inter4[0:pk, h0:h1], s0=CH0, s1=CH1, imm2=1.0,
                )
            for h0, h1 in HALVES:
                dgx_ps = spsum.tile([P, HALF], F32, tag="stream", name="dgx_ps")
                for c0, c1 in H_CHUNKS:
                    nc.tensor.matmul(dgx_ps[0:pk, c0:c1], lhsT=identm1[0:pk, 0:pk],
                                     rhs=ox[0:pk, h0 + c0 : h0 + c1],
                                     start=True, stop=False)
                    nc.tensor.matmul(dgx_ps[0:pk, c0:c1], lhsT=slabT[32:34, k, 0:pk],
                                     rhs=rows96[32:34, h0 + c0 : h0 + c1],
                                     start=False, stop=True)
                nc.scalar.activation(out=sqx[0:pk, h0:h1], in_=dgx_ps[0:pk, :], func=AF.Square)
            for h0, h1 in HALVES:
                dgy_ps = spsum.tile([P, HALF], F32, tag="stream", name="dgy_ps")
                for c0, c1 in H_CHUNKS:
                    nc.tensor.matmul(dgy_ps[0:pk, c0:c1], lhsT=identm1[0:pk, 0:pk],
                                     rhs=oy[0:pk, h0 + c0 : h0 + c1],
                                     start=True, stop=False)
                    nc.tensor.matmul(dgy_ps[0:pk, c0:c1], lhsT=slabT[64:66, k, 0:pk],
                                     rhs=rows96[64:66, h0 + c0 : h0 + c1],
                                     start=False, stop=True)
                nc.scalar.activation(out=sqy[0:pk, h0:h1], in_=dgy_ps[0:pk, :], func=AF.Square)

            # --- center distance + penalty ---
            cd = mt16("cd")
            nc.vector._custom_dve(
                CD_ANT, out=cd[0:pk, :], in0=Rcxb[0:pk, :], in1=Rcyb[0:pk, :],
                s0=qcx8[0:pk, sl], s1=qcy8[0:pk, sl], imm2=0.25,
            )
            diag = mt16("diag")
            nc.gpsimd.tensor_tensor(out=diag[0:pk, :], in0=sqx[0:pk, :], in1=sqy[0:pk, :], op=OP.add)
            pen = mt16("pen")
            nc.vector._custom_dve(
                DIVS_ANT, out=pen[0:pk, :], in0=diag[0:pk, :], in1=cd[0:pk, :],
                s0=CH0, s1=CH1, imm2=1.0,
            )

            # --- v / alpha*v ---
            v16 = mt16("v16")
            nc.scalar.activation(
                out=v16[0:pk, :], in_=Ab[0:pk, :], func=AF.Square, bias=na8[0:pk, sl]
            )
            vsq16 = mt16("vsq16")
            nc.scalar.activation(out=vsq16[0:pk, :], in_=v16[0:pk, :], func=AF.Square)
            adn = mt16("adn")
            nc.gpsimd.tensor_tensor(out=adn[0:pk, :], in0=niou[0:pk, :], in1=v16[0:pk, :], op=OP.add)
            av = mt16("av")
            nc.vector._custom_dve(
                DIVA_ANT, out=av[0:pk, :], in0=adn[0:pk, :], in1=vsq16[0:pk, :],
                s0=CH0, s1=CH1, imm2=1.0 + ADEN_EPS,
            )

            # --- final accumulation + output ---
            for n0, n1 in N_CHUNKS:
                nc.tensor.matmul(g[0:pk, n0:n1], lhsT=ident_h[0:pk, 0:pk], rhs=niou[0:pk, n0:n1],
                                 start=False, stop=False)
            for n0, n1 in N_CHUNKS:
                nc.tensor.matmul(g[0:pk, n0:n1], lhsT=ident_h[0:pk, 0:pk], rhs=pen[0:pk, n0:n1],
                                 start=False, stop=False)
            for n0, n1 in N_CHUNKS:
                nc.tensor.matmul(g[0:pk, n0:n1], lhsT=ident_h[0:pk, 0:pk], rhs=av[0:pk, n0:n1],
                                 start=False, stop=True)
            ost = ostage.tile([P, T], F16, tag="ost", name="ost")
            for h0, h1 in HALVES:
                nc.scalar.copy(out=ost[0:pk, h0:h1], in_=g[0:pk, h0:h1])
                nc.sync.dma_start(
                    out=bass.AP(
                        tensor=out_h[:, :].tensor,
                        offset=out_h[:, :].offset + k * P * T + h0,
                        ap=[[T, pk], [1, h1 - h0]],
                    ),
                    in_=ost[0:pk, h0:h1],
                )

    nc.compile()
    return nc


_NC_CACHE = None


def _get_nc():
    global _NC_CACHE
    if _NC_CACHE is None:
        _NC_CACHE = build_kernel()
    return _NC_CACHE


def kernel(pred_logits, pred_bbox, tgt_ids, tgt_bbox, **_unused):
    pred_logits = np.ascontiguousarray(np.asarray(pred_logits, dtype=np.float32))
    pred_bbox = np.ascontiguousarray(np.asarray(pred_bbox, dtype=np.float32))
    tgt_bbox = np.ascontiguousarray(np.asarray(tgt_bbox, dtype=np.float32))
    tid = np.ascontiguousarray(np.asarray(tgt_ids).astype(np.int32))

    nc = _get_nc()
    in_maps = [
        {
            "logits": pred_logits[i],
            "qbox": pred_bbox[i],
            "tbox": tgt_bbox,
            "tid": tid,
            "rep_marker": np.zeros(KVER + REPEAT, np.float32),
        }
        for i in range(B)
    ]
    res = run_bass_kernel_spmd(nc, in_maps, list(range(B)))
    out = np.stack([res.results[i]["out"] for i in range(B)], axis=0)
    return out.astype(np.float32)


if __name__ == "__main__":
    nc = build_kernel()
    print("v3 built OK")



# revision 37
# speedup vs baseline: 1.1059x; 1.1059x over previous
"""Trainium2 Bass kernel v4: engine-rebalanced CIoU cost matrix.

cost[q,t] = L1 + (-prob) + (-iou) + pen + alpha*v, per (query, target) pair.

Layout: queries on partitions (7 full tiles of 128 + repacked 4-query tail),
targets on the free axis (T=1600). Per-core batch-parallel over 8 cores.

v4 structural changes vs v3 (all driven by the CoreSim cost model, where a
custom-DVE op costs the same regardless of body complexity, Pool = 1333ns,
ACT ~ 1703ns, PE pass = 667ns per full [128,1600] tile):
- primary DVE customs per tile: wx=relu(qx2-tx2)+relu(tx1-qx1) (and wy),
  inter4=4*relu(qw-wx)*relu(qh-wy) (fused relu-product),
  rdiag=1/(cw^2+ch^2+eps) from the PE-built cw/ch psums,
  av=v^2/(1+eps+niou+v) (fused vsq+adn+divide).
- cw=wx+tw, ch=wy+th ride the PE (identity@wx + ones-row@tw).
- nun/niou: Pool stt + ACT Reciprocal + Pool mult (no psum stream).
- cd: ACT squares-with-bias from broadcast center rows; pen: Pool mults.
- L1 = 2wx+2wy+(tw+th)-(qw+qh) folds into the class matmul pass (+2I@wx/wy).
- f32 traw broadcast (9.9us DMA) replaced by four f16 coordinate rows.
- output copy psum->sbuf moved to Pool.
"""

import math
from contextlib import ExitStack

import numpy as np

import concourse.bass as bass
import concourse.bacc as bacc
import concourse.mybir as mybir
import concourse.tile as tile
import concourse.dve_ops as dve_ops
from concourse.bass_utils import run_bass_kernel_spmd
from concourse.dve_ops import DveOp, RECIP_APPROX_FAST_CONSTS
from concourse.dve_spec import (
    AluOp, Bin, C0, C1, C2, Spec, Src0, Src1, lower,
    _has_src1 as _hs1, maxx, minn, relu, sq,
)
from concourse.dve_uop import DveOpSpec
from concourse.masks import make_identity

B, Q, C, T = 8, 900, 92, 1600
REPEAT = 1
KVER = 42
EPS = 1e-6
ADEN_EPS = 1e-4  # keeps the alpha denominator fp16-normal; error << tolerance
P = 128
NQT = (Q + P - 1) // P
NFULL = NQT - 1
F32 = mybir.dt.float32
F16 = mybir.dt.float16
I32 = mybir.dt.int32
AF = mybir.ActivationFunctionType
OP = mybir.AluOpType
AX = mybir.AxisListType

N_CHUNKS = [(0, 512), (512, 1024), (1024, 1536), (1536, 1600)]
HALF = 800
HALVES = [(0, HALF), (HALF, T)]
H_CHUNKS = [(0, 512), (512, 800)]
TQ, TC, TW = 4, 32, 50
TWP = 64
Q0 = Q - TQ

CH0 = RECIP_APPROX_FAST_CONSTS["s0"]
CH1 = RECIP_APPROX_FAST_CONSTS["s1"]

# ---------------- custom DVE op registration (idempotent) -------------------


def _np_recip1(x, c0, c1):
    x32 = np.asarray(x, np.float32)
    nx = (~x32.view(np.int32)).view(np.float32)
    y0 = nx * c0
    return y0 * (c1 - x32 * y0)


def _recip1(x):
    nx = Bin(AluOp.BITWISE_NOT, x, x)
    y0 = nx * C0
    return y0 * (C1 - x * y0)


def _mk_op(name, body, reference):
    spec = Spec(body=body, reference=reference)
    ver = "v3"
    tmp = DveOpSpec(name=name, opcode=1, uops=lower(spec, ver=ver), rd1_en=_hs1(spec))
    return DveOp(name, spec, False, {ver: tmp.sha(ver)})


_OPS = {}


def _register_ops():
    global WX_ANT, INT4_ANT, DIVS_ANT, AV_ANT, ATAN_ANT
    defs = [
        # wx = relu(qx2 - tx2) + relu(tx1 - qx1) = convex_width - tw
        ("WX_ANT4", relu(C0 - Src0) + relu(Src1 - C1),
         lambda in0, in1, s0, s1, imm2: (
             np.maximum(s0 - in0, 0.0) + np.maximum(in1 - s1, 0.0))),
        # inter4 = imm * relu(qw - wx) * relu(qh - wy)
        ("INT4_ANT4", relu(C0 - Src0) * relu(C1 - Src1) * C2,
         lambda in0, in1, s0, s1, imm2: (
             np.maximum(s0 - in0, 0.0) * np.maximum(s1 - in1, 0.0) * imm2)),
        # out = in1 / in0 * imm  (one-Newton reciprocal; v3-proven op)
        ("DIVS_ANT3", Src1 * _recip1(Src0) * C2,
         lambda in0, in1, s0, s1, imm2: in1 * _np_recip1(in0, s0, s1) * imm2),
        # av = v^2 / (adn0 + imm), adn0 = niou + v built on Pool, imm = 1+eps
        ("AV_ANT4", sq(Src1) * _recip1(Src0 + C2),
         lambda in0, in1, s0, s1, imm2: (
             np.asarray(in1, np.float32) ** 2
             * _np_recip1(np.asarray(in0, np.float32) + imm2, s0, s1))),
        # atan(min(in0, in1)) on [0,1]: 3-term minimax poly m*(c0+c1 m^2+c2 m^4),
        # max err ~6e-4 rad. in0 = r, in1 = 1/r (range reduction folded in).
        # Keeps Arctan off the ACT table (one table set serves the kernel).
        ("ATAN_ANT4", (lambda m: (lambda t: m * (C0 + (C1 + C2 * t) * t))(sq(m)))(minn(Src0, Src1)),
         lambda in0, in1, s0, s1, imm2: (
             np.minimum(np.asarray(in0, np.float32), np.asarray(in1, np.float32))
             * (s0 + (s1 + imm2 * np.minimum(np.asarray(in0, np.float32), np.asarray(in1, np.float32)) ** 2)
                * np.minimum(np.asarray(in0, np.float32), np.asarray(in1, np.float32)) ** 2))),
    ]
    for name, body, ref in defs:
        if name in dve_ops._SUB_OPCODE_FOR_NAME:
            _OPS[name] = next(o for o in dve_ops.OPS if o.name == name)
            continue
        op = _mk_op(name, body, ref)
        row = dve_ops._CUSTOM_DVE_ROW_BASE + len(dve_ops.OPS)
        assert row < 0x20, "custom-DVE 5-bit row overflow"
        dve_ops.OPS.append(op)
        dve_ops._SUB_OPCODE_FOR_NAME[name] = row
        dve_ops.CUSTOM_DVE_SPECS[name] = op.spec
        _OPS[name] = op
    WX_ANT = _OPS["WX_ANT4"]
    INT4_ANT = _OPS["INT4_ANT4"]
    DIVS_ANT = _OPS["DIVS_ANT3"]
    AV_ANT = _OPS["AV_ANT4"]
    ATAN_ANT = _OPS["ATAN_ANT4"]


_register_ops()

# ---------------------------------------------------------------------------


def _bcast_ap(ap, npart, inner_ap):
    return bass.AP(tensor=ap.tensor, offset=ap.offset, ap=[[0, npart]] + inner_ap)


def emit_atan(nc, dst, wt, ht, mkt):
    """atan(wt/(ht+eps)) with range reduction; mkt() yields scratch tiles."""
    t1 = mkt()
    nc.vector.tensor_scalar(out=t1, in0=ht, scalar1=EPS, scalar2=None, op0=OP.add)
    t2 = mkt()
    nc.vector.reciprocal_approx_fast(out=t2, in_=t1)
    r = mkt()
    nc.vector.tensor_tensor(out=r, in0=wt, in1=t2, op=OP.mult)
    ri = mkt()
    nc.vector.reciprocal_approx_fast(out=ri, in_=r)
    atc = mkt()
    nc.vector._custom_dve(
        ATAN_ANT, out=atc, in0=r, in1=ri,
        s0=0.9953538, s1=-0.2886858, imm2=0.0793312,
    )
    m = mkt()
    nc.vector.tensor_scalar(out=m, in0=r, scalar1=1.0, scalar2=None, op0=OP.is_gt)
    t3 = mkt()
    nc.vector.tensor_scalar(
        out=t3, in0=atc, scalar1=-2.0, scalar2=math.pi / 2.0, op0=OP.mult, op1=OP.add
    )
    mt = mkt()
    nc.vector.tensor_tensor(out=mt, in0=m, in1=t3, op=OP.mult)
    nc.vector.tensor_tensor(out=dst, in0=atc, in1=mt, op=OP.add)


def build_kernel():
    nc = bacc.Bacc()

    logits_h = nc.declare_dram_parameter("logits", [Q, C], F32, isOutput=False)
    qbox_h = nc.declare_dram_parameter("qbox", [Q, 4], F32, isOutput=False)
    tbox_h = nc.declare_dram_parameter("tbox", [T, 4], F32, isOutput=False)
    tid_h = nc.declare_dram_parameter("tid", [T], I32, isOutput=False)
    nc.declare_dram_parameter("rep_marker", [KVER + REPEAT], F32, isOutput=False)
    out_h = nc.declare_dram_parameter("out", [Q, T], F16, isOutput=True)

    with ExitStack() as ctx:
        tc = ctx.enter_context(tile.TileContext(nc))
        consts = ctx.enter_context(tc.tile_pool(name="consts", bufs=1))
        rows = ctx.enter_context(tc.tile_pool(name="rows", bufs=1))
        qcols = ctx.enter_context(tc.tile_pool(name="qcols", bufs=1))
        tailp = ctx.enter_context(tc.tile_pool(name="tailp", bufs=1))

        # ---------------- constants ----------------
        ident_h = consts.tile([P, P], F16, tag="ident_h")
        make_identity(nc, ident_h)
        identp2 = consts.tile([P, P], F16, tag="identp2")
        make_identity(nc, identp2)
        nc.vector.tensor_scalar(out=identp2, in0=identp2, scalar1=2.0, scalar2=None, op0=OP.mult)
        ones1 = consts.tile([33, P], F16, tag="ones1")
        nc.vector.memset(ones1, 1.0)
        ic_i = consts.tile([C, 1], I32, tag="ic_i")
        nc.gpsimd.iota(ic_i, pattern=[[0, 1]], base=0, channel_multiplier=1)
        ic_f = consts.tile([C, 1], F32, tag="ic_f")
        nc.vector.tensor_copy(ic_f, ic_i)

        # persistent row tiles
        # R01X: onehot rows 0..91, row 92 = tw+th (L1 bilinear), row 93 = ones
        R01X = rows.tile([C + 2, T], F16, tag="R01X")
        # broadcast rows [128, T] f16 (tw/th consumed as matmul rhs at
        # partitions 0 / 32, so they ride the cheap broadcast DMA shape)
        Rx1b = rows.tile([P, T], F16, tag="Rx1b")
        Rx2b = rows.tile([P, T], F16, tag="Rx2b")
        Ry1b = rows.tile([P, T], F16, tag="Ry1b")
        Ry2b = rows.tile([P, T], F16, tag="Ry2b")
        Ra4b = rows.tile([P, T], F16, tag="Ra4b")
        Rcxb = rows.tile([P, T], F16, tag="Rcxb")
        Rcyb = rows.tile([P, T], F16, tag="Rcyb")
        Ab = rows.tile([P, T], F16, tag="Ab")
        twb = rows.tile([P, T], F16, tag="twb")
        thb = rows.tile([P, T], F16, tag="thb")

        eTX = qcols.tile([C + 2, NQT, P], F16, tag="eTX")

        # tail persistent tiles (prep written before main loop, read after)
        trawt = tailp.tile([P, TW, 4], F32, tag="trawt")
        Rw32t = tailp.tile([P, TW], F32, tag="Rw32t")
        Rh32t = tailp.tile([P, TW], F32, tag="Rh32t")
        RwHt = tailp.tile([P, TW], F16, tag="RwHt")
        RhHt = tailp.tile([P, TW], F16, tag="RhHt")
        Ra4t = tailp.tile([P, TW], F32, tag="Ra4t")
        Rcxt = tailp.tile([P, TW], F32, tag="Rcxt")
        Rcyt = tailp.tile([P, TW], F32, tag="Rcyt")
        Ratt = tailp.tile([P, TW], F32, tag="Ratt")
        tqb = tailp.tile([P, 4], F32, tag="tqb")
        g50 = tailp.tile([P, TW], F16, tag="g50")

        def ttile(tag):
            return tailp.tile([P, 1], F32, tag=tag, name=tag)

        tqw = ttile("tqw")
        tqh = ttile("tqh")
        tnqx1 = ttile("tnqx1")
        tnqy1 = ttile("tnqy1")
        tnqx2 = ttile("tnqx2")
        tnqy2 = ttile("tnqy2")
        tqa4e = ttile("tqa4e")
        tnqcx = ttile("tnqcx")
        tnqcy = ttile("tnqcy")
        tqat = ttile("tqat")
        tnqat = ttile("tnqat")

        mtiles = ctx.enter_context(tc.tile_pool(name="mtiles", bufs=2))
        ostage = ctx.enter_context(tc.tile_pool(name="ostage", bufs=2))

        with tc.tile_pool(name="scratch", bufs=1) as scratch, tc.tile_pool(
            name="prep_psum", bufs=2, space="PSUM"
        ) as ppsum:
            # ---- input DMAs issued first ----
            CP, CW = 100, 16
            ctb = scratch.tile([P, CW, 4], F32, tag="ctb")
            nc.vector.memset(ctb, 1.0)
            nc.vector.memset(ctb[:, :, 0:2], 0.25)
            nc.sync.dma_start(
                out=ctb[0:CP, :, :],
                in_=bass.AP(
                    tensor=tbox_h[:, :].tensor,
                    offset=tbox_h[:, :].offset,
                    ap=[[CW * 4, CP], [4, CW], [1, 4]],
                ),
            )
            tid_i = scratch.tile([C, T], I32, tag="tid_i")
            for (h0, h1), eng in zip(HALVES, (nc.gpsimd, nc.sync)):
                eng.dma_start(
                    out=tid_i[:, h0:h1],
                    in_=bass.AP(
                        tensor=tid_h[:].tensor, offset=tid_h[:].offset + h0,
                        ap=[[0, C], [1, h1 - h0]],
                    ),
                )
            qb = qcols.tile([P, NQT, 4], F32, tag="qb")
            nc.vector.memset(qb, 1.0)
            nc.vector.memset(qb[:, :, 0:2], 0.25)
            nfull = Q // P
            nc.gpsimd.dma_start(
                out=qb[:, 0:nfull, :],
                in_=bass.AP(
                    tensor=qbox_h[:, :].tensor,
                    offset=qbox_h[:, :].offset,
                    ap=[[4, P], [P * 4, nfull], [1, 4]],
                ),
            )
            nc.gpsimd.dma_start(out=qb[0 : Q - nfull * P, nfull, :], in_=qbox_h[nfull * P : Q, :])
            # logits: one batched DMA for the 7 full tiles + one for the tail
            L8 = qcols.tile([P, NQT, C], F32, tag="L8")
            nc.gpsimd.dma_start(
                out=L8[:, 0:nfull, :],
                in_=bass.AP(
                    tensor=logits_h[:, :].tensor,
                    offset=logits_h[:, :].offset,
                    ap=[[C, P], [P * C, nfull], [1, C]],
                ),
            )
            nc.gpsimd.dma_start(
                out=L8[0 : Q - nfull * P, nfull, :], in_=logits_h[nfull * P : Q, :]
            )

            # ---- compact target rows (atan-free part) ----
            def cs32(tag):
                return scratch.tile([P, CW], F32, tag=tag, name=tag)

            def cs16(tag):
                return scratch.tile([P, CW], F16, tag=tag, name=tag)

            cRw = cs32("cRw")
            cRh = cs32("cRh")
            nc.vector.tensor_tensor(out=cRw, in0=ctb[:, :, 2], in1=ctb[:, :, 0], op=OP.subtract)
            nc.vector.tensor_tensor(out=cRh, in0=ctb[:, :, 3], in1=ctb[:, :, 1], op=OP.subtract)
            c_twh = cs16("c_twh")
            nc.vector.tensor_tensor(out=c_twh, in0=cRw, in1=cRh, op=OP.add)
            c_ones = cs16("c_ones")
            nc.vector.memset(c_ones, 1.0)
            c_Rx1 = cs16("c_Rx1")
            c_Ry1 = cs16("c_Ry1")
            c_Rx2 = cs16("c_Rx2")
            c_Ry2 = cs16("c_Ry2")
            nc.vector.tensor_copy(c_Rx1, ctb[:, :, 0])
            nc.vector.tensor_copy(c_Ry1, ctb[:, :, 1])
            nc.vector.tensor_copy(c_Rx2, ctb[:, :, 2])
            nc.vector.tensor_copy(c_Ry2, ctb[:, :, 3])

            # ---- DRAM bounces (early: the corner rows gate tile 0) ----
            # drows: 0..3 = x1,x2,y1,y2  4 = A  10,11 = twh, ones
            drows = nc.dram_tensor("drows", [12, T], F16)
            for i, ctile in ((0, c_Rx1), (1, c_Rx2), (2, c_Ry1), (3, c_Ry2),
                             (10, c_twh), (11, c_ones)):
                eng = nc.sync if i % 2 == 0 else nc.gpsimd
                eng.dma_start(out=drows[i : i + 1, :], in_=ctile[0:CP, :])

            def bcast_in(i, btile, e0, e1):
                for (h0, h1), eng in zip(HALVES, (e0, e1)):
                    eng.dma_start(
                        out=btile[:, h0:h1],
                        in_=bass.AP(
                            tensor=drows[:, :].tensor,
                            offset=drows[:, :].offset + i * T + h0,
                            ap=[[0, P], [1, h1 - h0]],
                        ),
                    )

            bcast_in(1, Rx2b, nc.sync, nc.gpsimd)
            bcast_in(0, Rx1b, nc.gpsimd, nc.sync)
            bcast_in(3, Ry2b, nc.sync, nc.gpsimd)
            bcast_in(2, Ry1b, nc.gpsimd, nc.sync)
            nc.sync.dma_start(out=R01X[C : C + 2, :], in_=drows[10:12, :])
            # derived broadcast rows (one-time engine ops, off the DMA queues)
            nc.gpsimd.tensor_tensor(out=twb, in0=Rx2b, in1=Rx1b, op=OP.subtract)
            nc.gpsimd.tensor_tensor(out=thb, in0=Ry2b, in1=Ry1b, op=OP.subtract)
            nc.vector.tensor_tensor(out=Rcxb, in0=Rx1b, in1=Rx2b, op=OP.add)
            nc.vector.tensor_tensor(out=Rcyb, in0=Ry1b, in1=Ry2b, op=OP.add)
            nc.gpsimd.tensor_tensor(out=Ra4b, in0=twb, in1=thb, op=OP.mult)
            nc.gpsimd.tensor_scalar(
                out=Ra4b, in0=Ra4b, scalar1=4.0, scalar2=None, op0=OP.mult
            )

            # ---- softmax phase A ----
            mneg8 = qcols.tile([P, NQT], F32, tag="mneg8")
            ssum8 = qcols.tile([P, NQT], F32, tag="ssum8")
            nc.vector.memset(ssum8, 1.0)
            e_all = qcols.tile([P, NQT, C], F32, tag="e_all")
            for k in [NQT - 1] + list(range(NFULL)):
                pk = min(P, Q - k * P)
                nc.vector.tensor_reduce(
                    out=mneg8[0:pk, k : k + 1], in_=L8[0:pk, k, :], axis=AX.X, op=OP.max,
                    negate=True,
                )
                nc.scalar.activation(
                    out=e_all[0:pk, k, :], in_=L8[0:pk, k, :], func=AF.Exp,
                    bias=mneg8[0:pk, k : k + 1], scale=1.0,
                    accum_out=ssum8[0:pk, k : k + 1],
                )
            nr8 = qcols.tile([P, NQT], F32, tag="nr8")
            nc.vector.reciprocal(out=nr8, in_=ssum8)
            nc.vector.tensor_scalar(
                out=nr8, in0=nr8, scalar1=-1.0, scalar2=None, op0=OP.mult
            )

            # ---- onehot ----
            nc.vector.tensor_scalar(
                out=R01X[0:C, :], in0=tid_i, scalar1=ic_f[:, 0:1], scalar2=None,
                op0=OP.is_equal,
            )

            # ---- compact atan row + Ab bounce ----
            cAt = cs32("cAt")
            _atc = [0]

            def _mka():
                _atc[0] += 1
                return scratch.tile([P, CW], F32, tag="att", name="att", bufs=5)

            emit_atan(nc, cAt, cRw, cRh, _mka)
            c_A = cs16("c_A")
            nc.vector.tensor_scalar(
                out=c_A, in0=cAt, scalar1=2.0 / math.pi, scalar2=None, op0=OP.mult
            )
            nc.gpsimd.dma_start(out=drows[4:5, :], in_=c_A[0:CP, :])
            bcast_in(4, Ab, nc.sync, nc.gpsimd)

            # ---- per-query scalars ----
            qx1 = qb[:, :, 0]
            qy1 = qb[:, :, 1]
            qx2 = qb[:, :, 2]
            qy2 = qb[:, :, 3]

            def qt(tag):
                return qcols.tile([P, NQT], F32, tag=tag, name=tag)

            qw8 = qt("qw8")
            qh8 = qt("qh8")
            nc.vector.tensor_tensor(out=qw8, in0=qx2, in1=qx1, op=OP.subtract)
            nc.vector.tensor_tensor(out=qh8, in0=qy2, in1=qy1, op=OP.subtract)
            # qa4e = 4*qw*qh + 4*eps (positive, subtracted in the nun stt)
            qa4e8 = qt("qa4e8")
            nc.vector.scalar_tensor_tensor(
                out=qa4e8, in0=qw8, scalar=4.0, in1=qh8, op0=OP.mult, op1=OP.mult
            )
            nc.vector.tensor_scalar(
                out=qa4e8, in0=qa4e8, scalar1=4.0 * EPS, scalar2=None, op0=OP.add
            )
            # nqcx2 = -(qx1+qx2)/2 (ACT ex bias at scale 0.5); qcy8 = qy1+qy2
            nqcx28 = qt("nqcx28")
            nc.vector.tensor_tensor(out=nqcx28, in0=qx1, in1=qx2, op=OP.add)
            nc.vector.tensor_scalar(
                out=nqcx28, in0=nqcx28, scalar1=-0.5, scalar2=None, op0=OP.mult
            )
            qcy8 = qt("qcy8")
            nc.vector.tensor_tensor(out=qcy8, in0=qy1, in1=qy2, op=OP.add)
            # nqwqh = -(qw+qh) (eTX row C+1 weight)
            nqwqh8 = qt("nqwqh8")
            nc.vector.scalar_tensor_tensor(
                out=nqwqh8, in0=qw8, scalar=-1.0, in1=qh8, op0=OP.mult, op1=OP.subtract
            )
            na8 = qt("na8")
            qat8 = qt("qat8")
            _qtc = [0]

            def _mkq():
                _qtc[0] += 1
                return qcols.tile([P, NQT], F32, tag="qat_t", name="qat_t", bufs=5)

            emit_atan(nc, qat8, qw8, qh8, _mkq)
            nc.vector.tensor_scalar(
                out=na8, in0=qat8, scalar1=-2.0 / math.pi, scalar2=None, op0=OP.mult
            )

            # ---- tail input DMAs + scalar prep + atans ----
            for q in range(TQ):
                nc.sync.dma_start(
                    out=tqb[q * TC : (q + 1) * TC, :],
                    in_=bass.AP(
                        tensor=qbox_h[:, :].tensor,
                        offset=qbox_h[:, :].offset + (Q0 + q) * 4,
                        ap=[[0, TC], [1, 4]],
                    ),
                )
            for q in range(TQ):
                nc.sync.dma_start(
                    out=trawt[q * TC : (q + 1) * TC, :, :],
                    in_=bass.AP(
                        tensor=tbox_h[:, :].tensor,
                        offset=tbox_h[:, :].offset,
                        ap=[[TW * 4, TC], [4, TW], [1, 4]],
                    ),
                )
            _tat_tiles = [ttile(f"tat{i}") for i in range(9)]
            nc.vector.tensor_tensor(out=tqw, in0=tqb[:, 2:3], in1=tqb[:, 0:1], op=OP.subtract)
            nc.vector.tensor_tensor(out=tqh, in0=tqb[:, 3:4], in1=tqb[:, 1:2], op=OP.subtract)
            for dst, src in (
                (tnqx1, tqb[:, 0:1]), (tnqy1, tqb[:, 1:2]),
                (tnqx2, tqb[:, 2:3]), (tnqy2, tqb[:, 3:4]),
            ):
                nc.vector.tensor_scalar(out=dst, in0=src, scalar1=-1.0, scalar2=None, op0=OP.mult)
            nc.vector.scalar_tensor_tensor(
                out=tqa4e, in0=tqw, scalar=4.0, in1=tqh, op0=OP.mult, op1=OP.mult
            )
            nc.vector.tensor_scalar(
                out=tqa4e, in0=tqa4e, scalar1=4.0 * EPS, scalar2=None, op0=OP.add
            )
            nc.vector.scalar_tensor_tensor(
                out=tnqcx, in0=tqb[:, 0:1], scalar=-1.0, in1=tqb[:, 2:3], op0=OP.mult, op1=OP.subtract
            )
            nc.vector.scalar_tensor_tensor(
                out=tnqcy, in0=tqb[:, 1:2], scalar=-1.0, in1=tqb[:, 3:4], op0=OP.mult, op1=OP.subtract
            )
            _ttc = [0]

            def _mkt1():
                t = _tat_tiles[_ttc[0]]
                _ttc[0] += 1
                return t

            emit_atan(nc, tqat, tqw, tqh, _mkt1)
            nc.vector.tensor_scalar(
                out=tnqat, in0=tqat, scalar1=-2.0 / math.pi, scalar2=None, op0=OP.mult
            )
            ttx1 = trawt[:, :, 0]
            tty1 = trawt[:, :, 1]
            ttx2 = trawt[:, :, 2]
            tty2 = trawt[:, :, 3]
            nc.vector.tensor_tensor(out=Rw32t, in0=ttx2, in1=ttx1, op=OP.subtract)
            nc.vector.tensor_tensor(out=Rh32t, in0=tty2, in1=tty1, op=OP.subtract)
            nc.vector.tensor_copy(RwHt[:, :], Rw32t[:, :])
            nc.vector.tensor_copy(RhHt[:, :], Rh32t[:, :])
            nc.vector.scalar_tensor_tensor(
                out=Ra4t, in0=Rw32t, scalar=4.0, in1=Rh32t, op0=OP.mult, op1=OP.mult
            )
            nc.vector.tensor_tensor(out=Rcxt, in0=ttx1, in1=ttx2, op=OP.add)
            nc.vector.tensor_tensor(out=Rcyt, in0=tty1, in1=tty2, op=OP.add)
            _ttc2 = [0]

            def _mkt2():
                t = tailp.tile([P, TW], F32, tag="attw", name="attw", bufs=5)
                return t[0:P, 0:TW]

            emit_atan(nc, Ratt, Rw32t, Rh32t, _mkt2)

            # ---- softmax phase B ----
            for k in [NQT - 1] + list(range(NFULL)):
                pk = min(P, Q - k * P)
                es = scratch.tile([P, C + 2], F16, tag="es", name="es", bufs=2)
                nc.vector.tensor_scalar(
                    out=es[0:pk, 0:C], in0=e_all[0:pk, k, :],
                    scalar1=nr8[0:pk, k : k + 1], scalar2=None, op0=OP.mult,
                )
                nc.vector.memset(es[0:pk, C : C + 1], 1.0)
                nc.vector.tensor_copy(es[0:pk, C + 1 : C + 2], nqwqh8[0:pk, k : k + 1])
                tp = ppsum.tile([C + 2, P], F16, tag="tp", name="tp")
                nc.tensor.transpose(tp[:, 0:pk], es[0:pk, :], ident_h[0:pk, 0:pk])
                nc.scalar.copy(out=eTX[:, k, 0:pk], in_=tp[:, 0:pk])

            # ---- tail class term: [4, 1600] matmul -> DRAM -> [128, 50] ----
            g4 = ppsum.tile([P, T], F32, tag="g4", name="g4", bufs=1)
            for n0, n1 in N_CHUNKS:
                nc.tensor.matmul(
                    g4[0:TQ, n0:n1], lhsT=eTX[0:C, NFULL, 0:TQ], rhs=R01X[0:C, n0:n1],
                    start=True, stop=True,
                )
            gst = scratch.tile([P, T], F16, tag="gst")
            nc.scalar.copy(out=gst[0:TQ, :], in_=g4[0:TQ, :])
            gdram = nc.dram_tensor("tail_g", [TQ, T], F16)
            nc.sync.dma_start(out=gdram[:, :], in_=gst[0:TQ, :])
            for q in range(TQ):
                nc.sync.dma_start(
                    out=g50[q * TC : (q + 1) * TC, 0:TW],
                    in_=bass.AP(
                        tensor=gdram[:, :].tensor,
                        offset=gdram[:, :].offset + q * T,
                        ap=[[TW, TC], [1, TW]],
                    ),
                )

        # ---------------- main loop ----------------
        gpsum = ctx.enter_context(tc.tile_pool(name="gpsum", bufs=1, space="PSUM"))
        spsum = ctx.enter_context(tc.tile_pool(name="spsum", bufs=2, space="PSUM"))

        def mt16(tag, bufs=2):
            return mtiles.tile([P, T], F16, tag=tag, name=tag, bufs=bufs)

        def emit_tail():
            with tc.tile_pool(name="ttmp16", bufs=14) as ttmp16, tc.tile_pool(
                name="ttmp32", bufs=7
            ) as ttmp32, tc.tile_pool(name="tadd", bufs=6) as tadd:
                ttx1 = trawt[:, :, 0]
                tty1 = trawt[:, :, 1]
                ttx2 = trawt[:, :, 2]
                tty2 = trawt[:, :, 3]

                def t16(a, b, op, tg="t16"):
                    o = ttmp16.tile([P, TWP], F16, tag=tg, name=tg)
                    nc.vector.tensor_tensor(out=o[:, 0:TW], in0=a, in1=b, op=op)
                    return o[:, 0:TW]

                def act16(in_, func, bias=0.0, scale=1.0):
                    o = ttmp16.tile([P, TWP], F16, tag="a16", name="a16")
                    nc.scalar.activation(out=o[:, 0:TW], in_=in_, func=func, bias=bias, scale=scale)
                    return o[:, 0:TW]

                def t32(tag):
                    return ttmp32.tile([P, TWP], F32, tag="t32", name=tag)

                adx1 = act16(ttx1, AF.Abs, bias=tnqx1)
                adx2 = act16(ttx2, AF.Abs, bias=tnqx2)
                uX = t16(adx1, adx2, OP.add, tg="lng")
                ady1 = act16(tty1, AF.Abs, bias=tnqy1)
                ady2 = act16(tty2, AF.Abs, bias=tnqy2)
                uY = t16(ady1, ady2, OP.add, tg="lng")
                sxw = t16(RwHt[:, :], uX, OP.subtract)
                px = act16(sxw, AF.Relu, bias=tqw)
                syw = t16(RhHt[:, :], uY, OP.subtract)
                py = act16(syw, AF.Relu, bias=tqh)
                inter4t = t16(px, py, OP.mult)
                nun = t32("nun")
                nc.vector.scalar_tensor_tensor(
                    out=nun[:, 0:TW], in0=inter4t, scalar=tqa4e, in1=Ra4t,
                    op0=OP.subtract, op1=OP.subtract,
                )
                rnu = t32("rnu")
                nc.vector.reciprocal_approx_fast(out=rnu[:, 0:TW], in_=nun[:, 0:TW])
                niout = tadd.tile([P, TWP], F16, tag="ad", name="niout")
                nc.vector.tensor_tensor(out=niout[:, 0:TW], in0=inter4t, in1=rnu[:, 0:TW], op=OP.mult)
                cwx = t16(RwHt[:, :], uX, OP.add)
                sqcw = act16(cwx, AF.Square, bias=tqw)
                cwy = t16(RhHt[:, :], uY, OP.add)
                sqch = act16(cwy, AF.Square, bias=tqh)
                diagt = t32("diagt")
                nc.vector.scalar_tensor_tensor(
                    out=diagt[:, 0:TW], in0=sqcw, scalar=4.0 * EPS, in1=sqch,
                    op0=OP.add, op1=OP.add,
                )
                rd = t32("rd")
                nc.vector.reciprocal_approx_fast(out=rd[:, 0:TW], in_=diagt[:, 0:TW])
                ex = act16(Rcxt[:, :], AF.Square, bias=tnqcx)
                ey = act16(Rcyt[:, :], AF.Square, bias=tnqcy)
                cd4t = t16(ex, ey, OP.add)
                pent = tadd.tile([P, TWP], F16, tag="ad", name="pent")
                nc.vector.tensor_tensor(out=pent[:, 0:TW], in0=cd4t, in1=rd[:, 0:TW], op=OP.mult)
                vt = act16(Ratt[:, :], AF.Square, bias=tnqat, scale=2.0 / math.pi)
                adent = t32("adent")
                nc.vector.scalar_tensor_tensor(
                    out=adent[:, 0:TW], in0=niout[:, 0:TW], scalar=1.0 + EPS, in1=vt,
                    op0=OP.add, op1=OP.add,
                )
                ra = t32("ra")
                nc.vector.reciprocal_approx_fast(out=ra[:, 0:TW], in_=adent[:, 0:TW])
                vsqt = act16(vt, AF.Square)
                avt = tadd.tile([P, TWP], F16, tag="ad", name="avt")
                nc.vector.tensor_tensor(out=avt[:, 0:TW], in0=vsqt, in1=ra[:, 0:TW], op=OP.mult)

                # f32 accumulation on DVE (tiny at fd=50)
                s1 = t32("s1")
                nc.vector.tensor_tensor(out=s1[:, 0:TW], in0=niout[:, 0:TW], in1=pent[:, 0:TW], op=OP.add)
                s2 = t32("s2")
                nc.vector.tensor_tensor(out=s2[:, 0:TW], in0=s1[:, 0:TW], in1=avt[:, 0:TW], op=OP.add)
                s3 = t32("s3")
                nc.vector.tensor_tensor(out=s3[:, 0:TW], in0=s2[:, 0:TW], in1=uX, op=OP.add)
                s4 = t32("s4")
                nc.vector.tensor_tensor(out=s4[:, 0:TW], in0=s3[:, 0:TW], in1=uY, op=OP.add)
                ostt = tailp.tile([P, TWP], F16, tag="ostt")
                nc.vector.tensor_tensor(
                    out=ostt[:, 0:TW], in0=g50[:, 0:TW], in1=s4[:, 0:TW], op=OP.add
                )
                for q in range(TQ):
                    nc.gpsimd.dma_start(
                        out=bass.AP(
                            tensor=out_h[:, :].tensor,
                            offset=out_h[:, :].offset + (Q0 + q) * T,
                            ap=[[TW, TC], [1, TW]],
                        ),
                        in_=ostt[q * TC : (q + 1) * TC, 0:TW],
                    )

        _tail_emitted = [False]
        for k in [kk for _rep in range(REPEAT) for kk in range(NFULL)]:
            if k == 4 and not _tail_emitted[0]:
                emit_tail()
                _tail_emitted[0] = True
            pk = P
            sl = slice(k, k + 1)

            # --- DVE customs: wx, wy ---
            wx = mt16("wx", bufs=3)
            nc.vector._custom_dve(
                WX_ANT, out=wx[0:pk, :], in0=Rx2b[0:pk, :], in1=Rx1b[0:pk, :],
                s0=qb[0:pk, k, 2:3], s1=qb[0:pk, k, 0:1],
            )
            wy = mt16("wy", bufs=3)
            nc.vector._custom_dve(
                WX_ANT, out=wy[0:pk, :], in0=Ry2b[0:pk, :], in1=Ry1b[0:pk, :],
                s0=qb[0:pk, k, 3:4], s1=qb[0:pk, k, 1:2],
            )
            # inter4 = 4*relu(qw-wx)*relu(qh-wy)
            inter4 = mt16("inter4", bufs=3)
            nc.vector._custom_dve(
                INT4_ANT, out=inter4[0:pk, :], in0=wx[0:pk, :], in1=wy[0:pk, :],
                s0=qw8[0:pk, sl], s1=qh8[0:pk, sl], imm2=4.0,
            )

            # --- nun / niou (Pool TS+TT+divide; stt is not a Pool opcode) ---
            nun = mt16("nun")
            nc.gpsimd.tensor_scalar(
                out=nun[0:pk, :], in0=inter4[0:pk, :], scalar1=qa4e8[0:pk, sl],
                scalar2=None, op0=OP.subtract,
            )
            nc.gpsimd.tensor_tensor(
                out=nun[0:pk, :], in0=nun[0:pk, :], in1=Ra4b[0:pk, :], op=OP.subtract
            )
            niou = mt16("niou")
            nc.vector._custom_dve(
                DIVS_ANT, out=niou[0:pk, :], in0=nun[0:pk, :], in1=inter4[0:pk, :],
                s0=CH0, s1=CH1, imm2=1.0,
            )

            # --- PE cw/ch psums per half; ACT squares them to SBUF ---
            sqx = mt16("sqx")
            sqy = mt16("sqy")
            for h0, h1 in HALVES:
                cw_ps = spsum.tile([P, HALF], F32, tag="stream", name="cw_ps")
                for c0, c1 in H_CHUNKS:
                    nc.tensor.matmul(cw_ps[0:pk, c0:c1], lhsT=ident_h[0:pk, 0:pk],
                                     rhs=wx[0:pk, h0 + c0 : h0 + c1],
                                     start=True, stop=False)
                    nc.tensor.matmul(cw_ps[0:pk, c0:c1], lhsT=ones1[0:1, 0:pk],
                                     rhs=twb[0:1, h0 + c0 : h0 + c1],
                                     start=False, stop=True)
                nc.scalar.activation(out=sqx[0:pk, h0:h1], in_=cw_ps[0:pk, :], func=AF.Square)
            for h0, h1 in HALVES:
                ch_ps = spsum.tile([P, HALF], F32, tag="stream", name="ch_ps")
                for c0, c1 in H_CHUNKS:
                    nc.tensor.matmul(ch_ps[0:pk, c0:c1], lhsT=ident_h[0:pk, 0:pk],
                                     rhs=wy[0:pk, h0 + c0 : h0 + c1],
                                     start=True, stop=False)
                    nc.tensor.matmul(ch_ps[0:pk, c0:c1], lhsT=ones1[32:33, 0:pk],
                                     rhs=thb[32:33, h0 + c0 : h0 + c1],
                                     start=False, stop=True)
                nc.scalar.activation(out=sqy[0:pk, h0:h1], in_=ch_ps[0:pk, :], func=AF.Square)

            # --- center distance + penalty (cd split ACT/Pool) ---
            diag = mt16("diag")
            nc.gpsimd.tensor_tensor(out=diag[0:pk, :], in0=sqx[0:pk, :], in1=sqy[0:pk, :], op=OP.add)
            ex = mt16("ex")
            nc.scalar.activation(
                out=ex[0:pk, :], in_=Rcxb[0:pk, :], func=AF.Square,
                bias=nqcx28[0:pk, sl], scale=0.5,
            )
            dy = mt16("dy")
            nc.gpsimd.tensor_scalar(
                out=dy[0:pk, :], in0=Rcyb[0:pk, :], scalar1=qcy8[0:pk, sl],
                scalar2=0.5, op0=OP.subtract, op1=OP.mult,
            )
            cd = mt16("cd")
            nc.gpsimd.tensor_tensor(out=cd[0:pk, :], in0=dy[0:pk, :], in1=dy[0:pk, :], op=OP.mult)
            nc.gpsimd.tensor_tensor(out=cd[0:pk, :], in0=cd[0:pk, :], in1=ex[0:pk, :], op=OP.add)
            pen = mt16("pen")
            nc.vector._custom_dve(
                DIVS_ANT, out=pen[0:pk, :], in0=diag[0:pk, :], in1=cd[0:pk, :],
                s0=CH0, s1=CH1, imm2=1.0,
            )

            # --- v / alpha*v ---
            v16 = mt16("v16")
            nc.scalar.activation(
                out=v16[0:pk, :], in_=Ab[0:pk, :], func=AF.Square, bias=na8[0:pk, sl]
            )
            adn0 = mt16("adn0")
            nc.gpsimd.tensor_tensor(
                out=adn0[0:pk, :], in0=niou[0:pk, :], in1=v16[0:pk, :], op=OP.add
            )
            av = mt16("av")
            nc.vector._custom_dve(
                AV_ANT, out=av[0:pk, :], in0=adn0[0:pk, :], in1=v16[0:pk, :],
                s0=CH0, s1=CH1, imm2=1.0 + ADEN_EPS,
            )

            # --- PE g: class + L1(2wx+2wy+rows) + niou + pen + av ---
            g = gpsum.tile([P, T], F32, tag="g", name="g")
            for n0, n1 in N_CHUNKS:
                nc.tensor.matmul(g[0:pk, n0:n1], lhsT=eTX[:, k, 0:pk], rhs=R01X[:, n0:n1],
                                 start=True, stop=False)
            for n0, n1 in N_CHUNKS:
                nc.tensor.matmul(g[0:pk, n0:n1], lhsT=identp2[0:pk, 0:pk], rhs=wx[0:pk, n0:n1],
                                 start=False, stop=False)
            for n0, n1 in N_CHUNKS:
                nc.tensor.matmul(g[0:pk, n0:n1], lhsT=identp2[0:pk, 0:pk], rhs=wy[0:pk, n0:n1],
                                 start=False, stop=False)
            for n0, n1 in N_CHUNKS:
                nc.tensor.matmul(g[0:pk, n0:n1], lhsT=ident_h[0:pk, 0:pk], rhs=niou[0:pk, n0:n1],
                                 start=False, stop=False)
            for n0, n1 in N_CHUNKS:
                nc.tensor.matmul(g[0:pk, n0:n1], lhsT=ident_h[0:pk, 0:pk], rhs=pen[0:pk, n0:n1],
                                 start=False, stop=False)
            for n0, n1 in N_CHUNKS:
                nc.tensor.matmul(g[0:pk, n0:n1], lhsT=ident_h[0:pk, 0:pk], rhs=av[0:pk, n0:n1],
                                 start=False, stop=True)
            ost = ostage.tile([P, T], F16, tag="ost", name="ost")
            for h0, h1 in HALVES:
                nc.scalar.copy(out=ost[0:pk, h0:h1], in_=g[0:pk, h0:h1])
                nc.sync.dma_start(
                    out=bass.AP(
                        tensor=out_h[:, :].tensor,
                        offset=out_h[:, :].offset + k * P * T + h0,
                        ap=[[T, pk], [1, h1 - h0]],
                    ),
                    in_=ost[0:pk, h0:h1],
                )

    nc.compile()
    return nc


_NC_CACHE = None


def _get_nc():
    global _NC_CACHE
    if _NC_CACHE is None:
        _NC_CACHE = build_kernel()
    return _NC_CACHE


def kernel(pred_logits, pred_bbox, tgt_ids, tgt_bbox, **_unused):
    pred_logits = np.ascontiguousarray(np.asarray(pred_logits, dtype=np.float32))
    pred_bbox = np.ascontiguousarray(np.asarray(pred_bbox, dtype=np.float32))
    tgt_bbox = np.ascontiguousarray(np.asarray(tgt_bbox, dtype=np.float32))
    tid = np.ascontiguousarray(np.asarray(tgt_ids).astype(np.int32))

    nc = _get_nc()
    in_maps = [
        {
            "logits": pred_logits[i],
            "qbox": pred_bbox[i],
            "tbox": tgt_bbox,
            "tid": tid,
            "rep_marker": np.zeros(KVER + REPEAT, np.float32),
        }
        for i in range(B)
    ]
    res = run_bass_kernel_spmd(nc, in_maps, list(range(B)))
    out = np.stack([res.results[i]["out"] for i in range(B)], axis=0)
    return out.astype(np.float32)


if __name__ == "__main__":
    nc = build_kernel()
    print("v4 built OK")
